# revision 1
# baseline (speedup 1.0000x reference)
"""Trainium2 Bass kernel for nn_Decoder (Tacotron-style LSTM encoder/decoder).

Architecture (8 NeuronCores, data-parallel over batch N=64 -> 8/core):
  - Transposed ("World B") layout: hidden dim on SBUF partitions, (chunk,batch)
    on the free dim, so the h produced by the elementwise tail is directly the
    next step's matmul rhs (no per-step transposes).
  - Teacher forcing / layer chunking: input-side projections are batched into
    large matmuls per 64-step chunk; only h @ Whh.T stays per-step.
  - Decoder runs a 4-layer chunk-lagged wavefront inside shared For_i loops
    with staggered semaphore resets.
  - Weights / h / x-projections in bf16 (validated: ~2e-3 absmax rel err),
    cell state c and PSUM accumulation in fp32.
"""

import numpy as np
import ml_dtypes

H = 256
NMEL = 80
D_ENC = 512
NCORES = 8
NL = 8          # batch per core
C = 64          # chunk (time) size
F32 = None      # set after mybir import (lazy, so numpy-side helpers work alone)

_prog_cache = {}


def _build_program(S, T):
    """Build the Bass program for full sequence length S (encoder) and T
    (mels length; decoder runs TD = T padded steps). Returns (nc, meta)."""
    import concourse.bass as bass
    import concourse.mybir as mybir
    import concourse.tile as tile
    from concourse import bacc
    from concourse.bass import ds
    from concourse.masks import make_identity
    from contextlib import ExitStack

    BF = mybir.dt.bfloat16
    FP = mybir.dt.float32

    TD = T  # decoder steps padded to multiple of C (T=768 = 12*64; real steps T-1)
    assert S % C == 0 and TD % C == 0
    SC = S // C   # encoder chunks
    DC = TD // C  # decoder chunks

    nc = bacc.Bacc("TRN2", target_bir_lowering=False, debug=False,
                   num_devices=NCORES)

    # ---------------- DRAM I/O ----------------
    d_encrhs = nc.dram_tensor("encrhs", [4, 128, S, NL], BF, kind="ExternalInput").ap()
    d_mels = nc.dram_tensor("mels", [NL, NMEL, T], FP, kind="ExternalInput").ap()
    d_ictx = nc.dram_tensor("ictx", [NMEL], FP, kind="ExternalInput").ap()
    # pre-tiled bf16 weights (host-prepped)
    d_ewih = nc.dram_tensor("ewih", [128, 128, 128], BF, kind="ExternalInput").ap()
    d_ewhh = nc.dram_tensor("ewhh", [128, 64, 128], BF, kind="ExternalInput").ap()
    d_eb = nc.dram_tensor("eb", [1, 32, 128], BF, kind="ExternalInput").ap()
    d_dwih0 = nc.dram_tensor("dwih0", [97, 8, 128], BF, kind="ExternalInput").ap()
    d_dwih = nc.dram_tensor("dwih", [128, 48, 128], BF, kind="ExternalInput").ap()
    d_db = nc.dram_tensor("db", [1, 24, 128], BF, kind="ExternalInput").ap()
    d_dwhh = nc.dram_tensor("dwhh", [128, 64, 128], BF, kind="ExternalInput").ap()
    d_fcw = nc.dram_tensor("fcw", [128, 2, NMEL], BF, kind="ExternalInput").ap()
    d_fcb = nc.dram_tensor("fcb", [1, NMEL], BF, kind="ExternalInput").ap()
    d_out = nc.dram_tensor("out", [NL, NMEL, T], FP, kind="ExternalOutput").ap()
    out_r = d_out.rearrange("n c t -> c n t")  # [80, NL, T]

    CB = C * NL  # tokens per chunk = 512

    with tile.TileContext(nc) as tc:
        with ExitStack() as ctx:
            persist = ctx.enter_context(tc.tile_pool(name="persist", bufs=1))
            psum_x = ctx.enter_context(
                tc.tile_pool(name="psx", bufs=2, space="PSUM"))
            stash = ctx.enter_context(tc.tile_pool(name="stash", bufs=2))
            psum_g = None  # rebound per encoder/decoder scope

            ident = persist.tile([128, 128], BF)
            make_identity(nc, ident)
            ones = persist.tile([1, CB], BF)
            nc.vector.memset(ones, 1.0)

            # decoder init states copied out of encoder scope
            hinit = [persist.tile([128, 2 * NL], BF, tag=f"hi{l}", name=f"hinit{l}") for l in range(4)]
            cinit = [persist.tile([128, 2 * NL], FP, tag=f"ci{l}", name=f"cinit{l}") for l in range(4)]

            # ---------- one LSTM step (emitted inside a For_i body) ----------
            def emit_step(sc, k, i):
                """sc: scan dict; k: chunk idx; i: For_i loop var."""
                t0 = k * C
                if sc["fwd"]:
                    lt = i                      # local index into x-tilde chunk
                    rslot = t0 + i              # read slot (holds h_{t-1})
                    wslot = t0 + i + 1
                else:
                    lt = (C - 1) - i
                    rslot = sc["S"] - t0 - i    # = t+1
                    wslot = sc["S"] - 1 - t0 - i  # = t
                gp = sc["psum"].tile([128, 8 * NL], FP, tag=f"g_{sc['tag']}",
                                     bufs=sc.get("gbufs", 1))
                if sc.get("bt"):
                    xs = sc["xsb"][:, :, :, ds(lt, 1)]
                else:
                    xs = sc["xsb"][:, :, ds(lt, 1), :]
                nc.tensor.matmul(gp, ident, xs, start=True, stop=False)
                whh = sc["whh"]  # (sbuf tile, base index) pairs
                for kk in range(2):
                    rh = sc["hseq"][:, ds(rslot, 1), kk, :]
                    for j in range(8):
                        nc.tensor.matmul(
                            gp[:, j * NL:(j + 1) * NL],
                            whh[0][:, whh[1] + kk * 8 + j, :],
                            rh, start=False, stop=(kk == 1 and j == 7))
                sif = stash.tile([128, 4 * NL], FP, tag=f"sif_{sc['tag']}")
                tg = stash.tile([128, 2 * NL], FP, tag=f"tg_{sc['tag']}")
                so = stash.tile([128, 2 * NL], FP, tag=f"so_{sc['tag']}")
                tcl = stash.tile([128, 2 * NL], FP, tag=f"tc_{sc['tag']}")
                t1 = stash.tile([128, 2 * NL], FP, tag=f"t1_{sc['tag']}")
                t2 = stash.tile([128, 2 * NL], FP, tag=f"t2_{sc['tag']}")
                A = mybir.ActivationFunctionType
                nc.scalar.activation(sif, gp[:, 0:4 * NL], A.Sigmoid)
                nc.scalar.activation(tg, gp[:, 4 * NL:6 * NL], A.Tanh)
                nc.scalar.activation(so, gp[:, 6 * NL:8 * NL], A.Sigmoid)
                cst = sc["c"]
                nc.vector.tensor_mul(t1, sif[:, 0:2 * NL], tg)
                nc.vector.tensor_mul(t2, sif[:, 2 * NL:4 * NL], cst)
                nc.vector.tensor_add(cst, t1, t2)
                nc.scalar.activation(tcl, cst, A.Tanh)
                hw = sc["hseq"][:, ds(wslot, 1), :, :]
                nc.vector.tensor_mul(hw, so, tcl)

            # ---------- x-tilde chunk boundary (batched input projection) ----
            def emit_xtilde(sc, k):
                """Compute x-tilde for chunk k of scan sc into sc['xsb']."""
                nk = len(sc["xsrc"])
                for j in range(8):
                    xp = psum_x.tile([128, CB], FP, tag="xp")
                    for kk in range(nk):
                        rhs = sc["xsrc"][kk](k)
                        nc.tensor.matmul(xp, sc["wih"][0][:, sc["wih"][1] + kk * 8 + j, :]
                                         if sc["wih"][2] else sc["wih"][0][:, j, :],
                                         rhs, start=(kk == 0),
                                         stop=False if sc["brow"] else (kk == nk - 1))
                    if sc["brow"]:
                        nc.tensor.matmul(xp, sc["brow"][0][:, sc["brow"][1] + j, :],
                                         ones, start=False, stop=True)
                    dst = sc["xsb"][:, j, :, :]
                    if j < 4:
                        nc.scalar.copy(dst, xp)
                    else:
                        nc.vector.tensor_copy(dst, xp)

            # =======================================================
            # ENCODER
            # =======================================================
            with ExitStack() as ectx:
                epool = ectx.enter_context(tc.tile_pool(name="enc", bufs=1))
                psg_e = ectx.enter_context(
                    tc.tile_pool(name="psge", bufs=1, space="PSUM"))
                ew_ih = epool.tile([128, 128, 128], BF)
                ew_hh = epool.tile([128, 64, 128], BF)
                ew_b = epool.tile([1, 32, 128], BF)
                nc.sync.dma_start(out=ew_ih, in_=d_ewih)
                nc.sync.dma_start(out=ew_hh, in_=d_ewhh)
                nc.sync.dma_start(out=ew_b, in_=d_eb)

                # encoder input (host pre-transposed): [128, 4(k), S, NL]
                eo_bf = epool.tile([128, 4, S, NL], BF)
                for kk in range(4):
                    nc.sync.dma_start(out=eo_bf[:, kk], in_=d_encrhs[kk])

                escan = {}
                for (l, d) in [(0, 0), (0, 1), (1, 0), (1, 1)]:
                    tag = f"e{l}{d}"
                    hseq = epool.tile([128, S + 1, 2, NL], BF, tag=f"hs_{tag}")
                    cst = epool.tile([128, 2 * NL], FP, tag=f"c_{tag}")
                    xsb = epool.tile([128, 8, C, NL], BF, tag=f"x_{tag}")
                    nc.vector.memset(cst, 0.0)
                    init_slot = 0 if d == 0 else S
                    nc.vector.memset(hseq[:, init_slot], 0.0)
                    widx = ((l * 2 + d) * 2) * 8        # whh tile base
                    wxidx = ((l * 2 + d) * 4) * 8       # wih tile base
                    bidx = (l * 2 + d) * 8
                    if l == 0:
                        xsrc = []
                        for kk in range(4):
                            def f(k, kk=kk, d=d):
                                tr0 = k * C if d == 0 else S - (k + 1) * C
                                return eo_bf[:, kk, tr0:tr0 + C, :]
                            xsrc.append(f)
                    else:
                        xsrc = []
                        for kk in range(4):
                            def f(k, kk=kk, d=d):
                                tr0 = k * C if d == 0 else S - (k + 1) * C
                                if kk < 2:  # forward outputs of L0: slot t+1
                                    return escan["e00"]["hseq"][:, tr0 + 1:tr0 + C + 1, kk, :]
                                else:       # backward outputs of L0: slot t
                                    return escan["e01"]["hseq"][:, tr0:tr0 + C, kk - 2, :]
                            xsrc.append(f)
                    escan[tag] = dict(
                        tag=tag, fwd=(d == 0), S=S, hseq=hseq, c=cst, xsb=xsb,
                        whh=(ew_hh, widx), wih=(ew_ih, wxidx, True),
                        brow=(ew_b, bidx), xsrc=xsrc, psum=psg_e)

                # L0 phases then L1 phases (fwd+bwd interleaved per phase)
                for l in range(2):
                    scans = [escan[f"e{l}0"], escan[f"e{l}1"]]
                    for k in range(SC):
                        for sc in scans:
                            emit_xtilde(sc, k)
                        with tc.For_i(0, C // 2, 1, staggered_reset=False) as i:
                            for u in range(2):
                                for sc in scans:
                                    emit_step(sc, k, i * 2 + u)

                # copy finals into persistent init tiles
                fin = [("e00", S, True), ("e01", 0, True),
                       ("e10", S, False), ("e11", 0, False)]
                for li, (tag, slot, _) in enumerate(fin):
                    nc.vector.tensor_copy(hinit[li], escan[tag]["hseq"][:, slot])
                    nc.vector.tensor_copy(cinit[li], escan[tag]["c"])

            # =======================================================
            # DECODER (4-layer chunk-lagged wavefront)
            # =======================================================
            with ExitStack() as dctx:
                dpool = dctx.enter_context(tc.tile_pool(name="dec", bufs=1))
                psg_d = dctx.enter_context(
                    tc.tile_pool(name="psgd", bufs=1, space="PSUM"))
                dw_ih0 = dpool.tile([97, 8, 128], BF)
                dw_ih = dpool.tile([128, 48, 128], BF)
                dw_b = dpool.tile([1, 24, 128], BF)
                dw_hh = dpool.tile([128, 64, 128], BF)
                fw = dpool.tile([128, 2, NMEL], BF)
                fb = dpool.tile([1, NMEL], BF)
                nc.sync.dma_start(out=dw_ih0, in_=d_dwih0)
                nc.sync.dma_start(out=dw_ih, in_=d_dwih)
                nc.sync.dma_start(out=dw_b, in_=d_db)
                nc.sync.dma_start(out=dw_hh, in_=d_dwhh)
                nc.sync.dma_start(out=fw, in_=d_fcw)
                nc.sync.dma_start(out=fb, in_=d_fcb)

                teach = dpool.tile([97, CB], BF)
                nc.vector.memset(teach, 0.0)
                nc.vector.memset(teach[96:97, :], 1.0)  # bias row
                mst = dpool.tile([NMEL, NL, C], FP)
                icst = dpool.tile([NMEL, 1], FP)

                dscan = []
                for l in range(4):
                    tag = f"d{l}"
                    hseq = dpool.tile([128, TD + 1, 2, NL], BF, tag=f"hs_{tag}")
                    cst = dpool.tile([128, 2 * NL], FP, tag=f"c_{tag}")
                    xsb = dpool.tile([128, 8, NL, C] if l == 0 else [128, 8, C, NL], BF, tag=f"x_{tag}", name=f"xsb_{tag}")
                    nc.vector.tensor_copy(hseq[:, 0], hinit[l])
                    nc.vector.tensor_copy(cst, cinit[l])
                    if l == 0:
                        wih = (dw_ih0, 0, False)
                        brow = None
                        xsrc = [lambda k: teach[:, :]]
                    else:
                        wih = (dw_ih, (l - 1) * 16, True)
                        brow = (dw_b, (l - 1) * 8)
                        xsrc = []
                        for kk in range(2):
                            def f(k, kk=kk, l=l):
                                t0 = k * C
                                return dscan[l - 1]["hseq"][:, t0 + 1:t0 + C + 1, kk, :]
                            xsrc.append(f)
                    dscan.append(dict(
                        tag=tag, fwd=True, S=TD, hseq=hseq, c=cst, xsb=xsb,
                        whh=(dw_hh, l * 16), wih=wih, brow=brow, xsrc=xsrc,
                        psum=psg_d, bt=(l == 0)))

                mels_r = d_mels.rearrange("n c t -> c n t")  # [80, NL, T]

                def fill_teacher(k):
                    t0 = k * C
                    # teacher[t] = ictx if t==0 else mels[t-1]; tokens (b, t)
                    if k == 0:
                        nc.sync.dma_start(out=icst, in_=d_ictx.rearrange("(c o) -> c o", o=1))
                        nc.vector.tensor_copy(
                            mst[:, :, 0], icst.to_broadcast((NMEL, NL)))
                        nc.sync.dma_start(
                            out=mst[:, :, 1:], in_=mels_r[:, :, 0:C - 1])
                    else:
                        nc.sync.dma_start(
                            out=mst, in_=mels_r[:, :, t0 - 1:t0 + C - 1])
                    nc.vector.tensor_copy(teach[0:NMEL, :], mst)

                def emit_fc(k):
                    t0 = k * C
                    fp = psum_x.tile([NMEL, CB], FP, tag="xp", name=f"fcp{k}")
                    for kk in range(2):
                        nc.tensor.matmul(
                            fp, fw[:, kk, :],
                            dscan[3]["hseq"][:, t0 + 1:t0 + C + 1, kk, :],
                            start=(kk == 0), stop=False)
                    nc.tensor.matmul(fp, fb, ones, start=False, stop=True)
                    fst = stash.tile([NMEL, NL, C], FP, tag="fst", name=f"fst{k}")
                    nc.scalar.copy(fst, fp.rearrange("p (t b) -> p b t", b=NL))
                    if k == DC - 1:
                        nc.vector.memset(fst[:, :, C - 1:C], 0.0)
                    nc.sync.dma_start(out=out_r[:, :, t0:t0 + C], in_=fst)

                # wavefront phases
                for p in range(DC + 3):
                    active = [l for l in range(4) if 0 <= p - l < DC]
                    for l in active:
                        if l == 0:
                            fill_teacher(p)
                        emit_xtilde(dscan[l], p - l)
                    with tc.For_i(0, C // 2, 1, staggered_reset=False) as i:
                        for u in range(2):
                            for l in active:
                                emit_step(dscan[l], p - l, i * 2 + u)
                    if 0 <= p - 4 < DC:
                        emit_fc(p - 4)

                # ---------------- FC tail (chunks not covered in-loop) ----
                fc_done = {p - 4 for p in range(DC + 3) if 0 <= p - 4 < DC}
                for k in range(DC):
                    if k not in fc_done:
                        emit_fc(k)

    nc.compile()
    return nc


def _host_prep(inputs):
    """Slice batch across cores + pre-tile/cast weights. Returns in_maps."""
    bf16 = ml_dtypes.bfloat16

    def tiles_T(w, kchunks, jchunks):
        # w: [4H, D] fp32 -> list over (k, j) of w.T tiles [128, 128] bf16
        wT = np.ascontiguousarray(w.T).astype(bf16)  # [D, 4H]
        out = np.zeros((kchunks, jchunks, 128, 128), bf16)
        for k in range(kchunks):
            for j in range(jchunks):
                out[k, j] = wT[k * 128:(k + 1) * 128, j * 128:(j + 1) * 128]
        return out

    enc_Wih = np.asarray(inputs["enc_Wih"], np.float32)
    enc_Whh = np.asarray(inputs["enc_Whh"], np.float32)
    enc_b = np.asarray(inputs["enc_b"], np.float32)
    dec_Wih0 = np.asarray(inputs["dec_Wih0"], np.float32)
    dec_Wih = np.asarray(inputs["dec_Wih"], np.float32)
    dec_Whh = np.asarray(inputs["dec_Whh"], np.float32)
    dec_b = np.asarray(inputs["dec_b"], np.float32)
    fc_W = np.asarray(inputs["fc_W"], np.float32)
    fc_b = np.asarray(inputs["fc_b"], np.float32)
    ictx = np.asarray(inputs["init_ctx"], np.float32).reshape(-1)

    ewih = np.zeros((2, 2, 4, 8, 128, 128), bf16)
    ewhh = np.zeros((2, 2, 2, 8, 128, 128), bf16)
    eb = np.zeros((2, 2, 8, 128), bf16)
    for l in range(2):
        for d in range(2):
            ewih[l, d] = tiles_T(enc_Wih[l, d], 4, 8)
            ewhh[l, d] = tiles_T(enc_Whh[l, d], 2, 8)
            eb[l, d] = enc_b[l, d].reshape(8, 128).astype(bf16)

    # dec layer0: [97, 8, 128]: rows 0:80 = Wih0.T j-block, rows 80:96 zero,
    # row 96 = bias (engine base-partition must be in {0,32,64,96})
    dwih0 = np.zeros((97, 8, 128), bf16)
    w0T = dec_Wih0.T.astype(bf16)  # [80, 1024]
    for j in range(8):
        dwih0[0:80, j] = w0T[:, j * 128:(j + 1) * 128]
        dwih0[96, j] = dec_b[0, j * 128:(j + 1) * 128].astype(bf16)

    dwih = np.zeros((3, 2, 8, 128, 128), bf16)
    db = np.zeros((3, 8, 128), bf16)
    for l in range(3):
        dwih[l] = tiles_T(dec_Wih[l], 2, 8)
        db[l] = dec_b[l + 1].reshape(8, 128).astype(bf16)
    dwhh = np.zeros((4, 2, 8, 128, 128), bf16)
    for l in range(4):
        dwhh[l] = tiles_T(dec_Whh[l], 2, 8)

    fcw = np.zeros((2, 128, NMEL), bf16)
    fWT = fc_W.T.astype(bf16)  # [256, 80]
    fcw[0] = fWT[0:128]
    fcw[1] = fWT[128:256]

    encout = np.asarray(inputs["encoder_outputs"], np.float32)
    mels = np.asarray(inputs["mels"], np.float32)
    N = encout.shape[0]
    nb = N // NCORES

    base = {
        "ictx": ictx,
        "ewih": np.ascontiguousarray(ewih.reshape(128, 128, 128).transpose(1, 0, 2)),
        "ewhh": np.ascontiguousarray(ewhh.reshape(64, 128, 128).transpose(1, 0, 2)),
        "eb": np.ascontiguousarray(eb.reshape(1, 32, 128)),
        "dwih0": dwih0,
        "dwih": np.ascontiguousarray(dwih.reshape(48, 128, 128).transpose(1, 0, 2)),
        "db": np.ascontiguousarray(db.reshape(1, 24, 128)),
        "dwhh": np.ascontiguousarray(dwhh.reshape(64, 128, 128).transpose(1, 0, 2)),
        "fcw": np.ascontiguousarray(fcw.transpose(1, 0, 2)),
        "fcb": fc_b.astype(bf16).reshape(1, NMEL),
    }
    S = encout.shape[1]
    in_maps = []
    for cid in range(NCORES):
        m = dict(base)
        eo = encout[cid * nb:(cid + 1) * nb]  # [nb, S, 512]
        m["encrhs"] = np.ascontiguousarray(
            eo.transpose(2, 1, 0).reshape(4, 128, S, nb).astype(bf16))
        m["mels"] = np.ascontiguousarray(mels[cid * nb:(cid + 1) * nb])
        in_maps.append(m)
    return in_maps


def kernel(encoder_outputs, mels, text_lengths, output_lengths,
           enc_Wih, enc_Whh, enc_b, dec_Wih0, dec_Wih, dec_Whh, dec_b,
           fc_W, fc_b, init_ctx):
    from concourse import bass_utils

    inputs = dict(encoder_outputs=encoder_outputs, mels=mels,
                  enc_Wih=enc_Wih, enc_Whh=enc_Whh, enc_b=enc_b,
                  dec_Wih0=dec_Wih0, dec_Wih=dec_Wih, dec_Whh=dec_Whh,
                  dec_b=dec_b, fc_W=fc_W, fc_b=fc_b, init_ctx=init_ctx)
    N, S, _ = np.asarray(encoder_outputs).shape
    T = np.asarray(mels).shape[2]
    key = (S, T)
    if key not in _prog_cache:
        _prog_cache[key] = _build_program(S, T)
    nc = _prog_cache[key]
    in_maps = _host_prep(inputs)
    res = bass_utils.run_bass_kernel_spmd(nc, in_maps, core_ids=list(range(NCORES)))
    nb = N // NCORES
    out = np.zeros((N, NMEL, T), np.float32)
    for cid in range(NCORES):
        out[cid * nb:(cid + 1) * nb] = res.results[cid]["out"]
    return (out,)

